# revision 4
# baseline (speedup 1.0000x reference)
"""BinaryLinear on 8 TRN2 NeuronCores.

out = sign(x) @ sign(weight).T ; x [8192, 4096] f32, weight [4096, 4096] f32.

Sharding (4x2 grid): x row-blocks of 2048 across 4 groups, weight
out_feature-blocks of 2048 across 2 groups. Core c = (mb, ob) =
(c // 2, c % 2). Each core receives k-major (pre-transposed) shards:
  xT [4096, 2048] = x[mb].T,  wT [4096, 2048] = w[ob].T
and computes out_shard [2048, 2048] = sign(x[mb]) @ sign(w[ob]).T.

Device kernel: sign-cast both operands f32 -> bf16 (+-1 exact; products
+-1, fp32 PSUM accumulation of <=4096 terms is exact), keep the signed
weight shard resident in SBUF (128 KiB/partition), stream x row-tiles
through TensorE.
"""

import numpy as np

import concourse.bass as bass
import concourse.mybir as mybir
import concourse.tile as tile
from concourse.bass_utils import run_bass_kernel_spmd
from concourse.vector_clock import ScopedClock, VectorClock

N, K, O = 8192, 4096, 4096
MB, OB = 4, 2  # shard grid
MSH, OSH = N // MB, O // OB  # 2048, 2048 per-core shard dims
KC = K // 128  # 32 k-chunks
MT = MSH // 128  # 16 m-tiles
NT = 512  # matmul moving free dim
OT = OSH // NT  # 4 o-tiles

F32 = mybir.dt.float32
BF16 = mybir.dt.bfloat16
SIGN = mybir.ActivationFunctionType.Sign


def _split_drain_and_barrier(self, tick_clock, wait_clock):
    # This walrus build rejects >1 sem wait on a Drain ("Too many sync
    # wait commands"); emit one single-wait drain per active proc lane.
    gc = tick_clock.global_clock
    n = len(gc)
    for p in range(n):
        if gc[p] > 0:
            sub = VectorClock([gc[q] if q == p else 0 for q in range(n)])
            d = self.nc.sync.drain()
            wait_clock.add_sem_waits(d.ins, ScopedClock({None: sub}))
    self.nc.all_engine_barrier()
    assert self.sems is not None
    popped = self.nc._tile_sem_poison_stack.pop()
    assert popped is self._sem_poison
    self.nc.clear_and_free_semaphores(list(self.sems.allocated().values()))
    self.nc.all_engine_barrier()


tile.TileContext._drain_and_barrier = _split_drain_and_barrier


def _split_multi_waits(nc):
    """Walrus here allows at most ONE sem wait per instruction. Engines
    dispatch their streams in order, so waiting on k sems at one
    instruction == k single-wait EVSEMs followed by the instruction
    (for DMACopy on a DGE ring the preceding EVSEM stalls the issuing
    engine before it enqueues the descriptor — conservative, correct).
    """
    import bass_rust

    n_split = 0
    for func in nc.m.functions:
        for bb in func.blocks:
            new = []
            for ins in bb.instructions:
                si = ins.sync_info
                waits = list(si.on_wait) if si is not None else []
                if len(waits) > 1:
                    for w in waits[:-1]:
                        n_split += 1
                        ev = mybir.InstEventSemaphore(
                            name=f"I-waitsplit-{n_split}",
                            ins=[],
                            outs=[],
                            engine=ins.engine,
                        )
                        ev.sync_info = bass_rust.SyncInfo(
                            on_wait=[w], on_update=[]
                        )
                        new.append(ev)
                    ins.sync_info = bass_rust.SyncInfo(
                        on_wait=[waits[-1]], on_update=list(si.on_update)
                    )
                new.append(ins)
            bb.instructions = new
    return n_split


def build():
    nc = bass.Bass()
    xT = nc.declare_dram_parameter("xT", [K, MSH], F32, isOutput=False)
    wT = nc.declare_dram_parameter("wT", [K, OSH], F32, isOutput=False)
    out = nc.declare_dram_parameter("out", [MSH, OSH], F32, isOutput=True)

    xT4 = xT.rearrange("(c p) m -> p c m", p=128)  # [128, KC, MSH]
    wT3 = wT.rearrange("(c p) o -> p c o", p=128)  # [128, KC, OSH]

    with tile.TileContext(nc) as tc:
        with (
            tc.tile_pool(name="wstage", bufs=2) as wstage,
            tc.tile_pool(name="wres", bufs=KC) as wres,
            tc.tile_pool(name="xstage", bufs=1) as xstage,
            tc.tile_pool(name="xbin", bufs=2) as xbin,
            tc.tile_pool(name="psum", bufs=8, space="PSUM") as psum,
            tc.tile_pool(name="outb", bufs=4) as outb,
        ):
            wb = [None] * KC

            def prep_w(kc):
                wf = wstage.tile([128, OSH], F32, tag="wf")
                nc.sync.dma_start(wf[:], wT3[:, kc, :])
                t = wres.tile([128, OSH], BF16, tag="wres")
                nc.scalar.activation(t[:], wf[:], SIGN)
                wb[kc] = t

            for mt in range(MT):
                xf = xstage.tile([128, KC, 128], F32, tag="xf")
                nc.sync.dma_start(xf[:], xT4[:, :, bass.ts(mt, 128)])
                xb = xbin.tile([128, KC, 128], BF16, tag="xb")
                nc.scalar.activation(xb[:], xf[:], SIGN)

                pss = [
                    psum.tile([128, NT], F32, tag="ps", name=f"ps{mt}_{i}")
                    for i in range(OT)
                ]
                for kc in range(KC):
                    if mt == 0:
                        prep_w(kc)  # interleave w prep with first tile's MMs
                    for ot in range(OT):
                        nc.tensor.matmul(
                            pss[ot][:],
                            xb[:, kc, :],
                            wb[kc][:, bass.ts(ot, NT)],
                            start=(kc == 0),
                            stop=(kc == KC - 1),
                        )
                for ot in range(OT):
                    ob = outb.tile([128, NT], F32, tag="ob")
                    nc.vector.tensor_copy(ob[:], pss[ot][:])
                    nc.sync.dma_start(
                        out[bass.ts(mt, 128), bass.ts(ot, NT)], ob[:]
                    )
    _split_multi_waits(nc)
    return nc


_CACHE = {}


def _run(in_maps, trace=False, **kwargs):
    if "nc" not in _CACHE:
        _CACHE["nc"] = build()
    return run_bass_kernel_spmd(
        _CACHE["nc"], in_maps, core_ids=list(range(8)), trace=trace, **kwargs
    )


def _shard(x, weight):
    in_maps = []
    for c in range(8):
        mb, ob = c // 2, c % 2
        in_maps.append(
            {
                "xT": np.ascontiguousarray(x[mb * MSH : (mb + 1) * MSH, :].T),
                "wT": np.ascontiguousarray(
                    weight[ob * OSH : (ob + 1) * OSH, :].T
                ),
            }
        )
    return in_maps


def _gather(results):
    out = np.empty((N, O), dtype=np.float32)
    for c in range(8):
        mb, ob = c // 2, c % 2
        out[mb * MSH : (mb + 1) * MSH, ob * OSH : (ob + 1) * OSH] = results[c][
            "out"
        ]
    return out


def kernel(x: np.ndarray, weight: np.ndarray) -> np.ndarray:
    x = np.asarray(x, dtype=np.float32)
    weight = np.asarray(weight, dtype=np.float32)
    res = _run(_shard(x, weight))
    return _gather(res.results)


# revision 5
# speedup vs baseline: 1.6895x; 1.6895x over previous
"""BinaryLinear on 8 TRN2 NeuronCores.

out = sign(x) @ sign(weight).T ; x [8192, 4096] f32, weight [4096, 4096] f32.

Sharding (4x2 grid): x row-blocks of 2048 across 4 groups, weight
out_feature-blocks of 2048 across 2 groups. Core c = (mb, ob) =
(c // 2, c % 2). Each core computes out_shard [2048, 2048] =
sign(x[mb]) @ sign(w[ob]).T.

Host-side layout prep (free for the device):
  xP [16, 128, 32, 128]: xP[mt, p, kc, m] = x_shard[mt*128 + m, kc*128 + p]
     -> each m-tile's operand block is one contiguous 2 MiB DMA with
        16 KiB-contiguous reads per partition.
  wT [4096, 2048] = w_shard.T (k-major; rows are 8 KiB contiguous).

Device kernel: sign-cast both operands f32 -> fp8e4 (+-1 exact; products
+-1, fp32 PSUM accumulation of <=4096 terms is exact), keep the signed
weight shard resident in SBUF (64 KiB/partition), stream x row-tiles
through TensorE with DoubleRow matmuls (K=256 per pass, 2 MAC/cell/cyc).
"""

import numpy as np

import concourse.bass as bass
import concourse.mybir as mybir
import concourse.tile as tile
from concourse.bass_utils import run_bass_kernel_spmd
from concourse.vector_clock import ScopedClock, VectorClock

N, K, O = 8192, 4096, 4096
MB, OB = 4, 2  # shard grid
MSH, OSH = N // MB, O // OB  # 2048, 2048 per-core shard dims
KC = K // 128  # 32 k-chunks
KP = KC // 2  # 16 double-row k-pairs
MT = MSH // 128  # 16 m-tiles
NT = 512  # matmul moving free dim (psum bank)
OT = OSH // NT  # 4 o-tiles

F32 = mybir.dt.float32
FP8 = mybir.dt.float8e4
SIGN = mybir.ActivationFunctionType.Sign
DR = mybir.MatmulPerfMode.DoubleRow


def _split_drain_and_barrier(self, tick_clock, wait_clock):
    # This walrus build rejects >1 sem wait on a Drain ("Too many sync
    # wait commands"); emit one single-wait drain per active proc lane.
    gc = tick_clock.global_clock
    n = len(gc)
    for p in range(n):
        if gc[p] > 0:
            sub = VectorClock([gc[q] if q == p else 0 for q in range(n)])
            d = self.nc.sync.drain()
            wait_clock.add_sem_waits(d.ins, ScopedClock({None: sub}))
    self.nc.all_engine_barrier()
    assert self.sems is not None
    popped = self.nc._tile_sem_poison_stack.pop()
    assert popped is self._sem_poison
    self.nc.clear_and_free_semaphores(list(self.sems.allocated().values()))
    self.nc.all_engine_barrier()


tile.TileContext._drain_and_barrier = _split_drain_and_barrier


def _split_multi_waits(nc):
    """Walrus here allows at most ONE sem wait per instruction. Engines
    dispatch their streams in order, so waiting on k sems at one
    instruction == k single-wait EVSEMs followed by the instruction
    (for DMACopy on a DGE ring the preceding EVSEM stalls the issuing
    engine before it enqueues the descriptor — conservative, correct).
    """
    import bass_rust

    n_split = 0
    for func in nc.m.functions:
        for bb in func.blocks:
            new = []
            for ins in bb.instructions:
                si = ins.sync_info
                waits = list(si.on_wait) if si is not None else []
                if len(waits) > 1:
                    for w in waits[:-1]:
                        n_split += 1
                        ev = mybir.InstEventSemaphore(
                            name=f"I-waitsplit-{n_split}",
                            ins=[],
                            outs=[],
                            engine=ins.engine,
                        )
                        ev.sync_info = bass_rust.SyncInfo(
                            on_wait=[w], on_update=[]
                        )
                        new.append(ev)
                    ins.sync_info = bass_rust.SyncInfo(
                        on_wait=[waits[-1]], on_update=list(si.on_update)
                    )
                new.append(ins)
            bb.instructions = new
    return n_split


def build():
    nc = bass.Bass()
    xP = nc.declare_dram_parameter("xP", [MT, 128, KC, 128], F32, isOutput=False)
    wT = nc.declare_dram_parameter("wT", [K, OSH], F32, isOutput=False)
    out = nc.declare_dram_parameter("out", [MSH, OSH], F32, isOutput=True)

    wT4 = wT.rearrange("(t j p) o -> p t j o", p=128, j=2)  # [128, KP, 2, OSH]

    with tile.TileContext(nc) as tc:
        with (
            tc.tile_pool(name="wstage", bufs=2) as wstage,
            tc.tile_pool(name="wres", bufs=KP) as wres,
            tc.tile_pool(name="xstage", bufs=2) as xstage,
            tc.tile_pool(name="xbin", bufs=2) as xbin,
            tc.tile_pool(name="psum", bufs=8, space="PSUM") as psum,
            tc.tile_pool(name="outb", bufs=4) as outb,
        ):
            wb = [None] * KP

            def prep_w(t):
                wf = wstage.tile([128, 2, OSH], F32, tag="wf", name=f"wf{t}")
                nc.sync.dma_start(wf[:], wT4[:, t, :, :])
                w8 = wres.tile([128, 2, OSH], FP8, tag="wres", name=f"w8_{t}")
                nc.scalar.activation(w8[:], wf[:], SIGN)
                wb[t] = w8

            for mt in range(MT):
                xf = xstage.tile([128, KC, 128], F32, tag="xf", name=f"xf{mt}")
                nc.sync.dma_start(xf[:], xP[mt, :, :, :])
                xb = xbin.tile([128, KC, 128], FP8, tag="xb", name=f"xb{mt}")
                nc.scalar.activation(xb[:], xf[:], SIGN)

                pss = [
                    psum.tile([128, NT], F32, tag="ps", name=f"ps{mt}_{i}")
                    for i in range(OT)
                ]
                for t in range(KP):
                    if mt == 0:
                        prep_w(t)  # interleave w prep with first tile's MMs
                    for ot in range(OT):
                        nc.tensor.matmul(
                            pss[ot][:],
                            xb[:, 2 * t : 2 * t + 2, :],
                            wb[t][:, :, bass.ts(ot, NT)],
                            start=(t == 0),
                            stop=(t == KP - 1),
                            perf_mode=DR,
                        )
                for ot in range(OT):
                    ob = outb.tile([128, NT], F32, tag="ob", name=f"ob{mt}_{ot}")
                    nc.vector.tensor_copy(ob[:], pss[ot][:])
                    nc.sync.dma_start(
                        out[bass.ts(mt, 128), bass.ts(ot, NT)], ob[:]
                    )
    _split_multi_waits(nc)
    return nc


_CACHE = {}


def _run(in_maps, trace=False, **kwargs):
    if "nc" not in _CACHE:
        _CACHE["nc"] = build()
    return run_bass_kernel_spmd(
        _CACHE["nc"], in_maps, core_ids=list(range(8)), trace=trace, **kwargs
    )


def _pack_x(x_shard):
    # [2048, 4096] -> [16, 128, 32, 128] with [mt, p, kc, m] indexing
    x4 = x_shard.reshape(MT, 128, KC, 128)  # [mt, m, kc, p]
    return np.ascontiguousarray(x4.transpose(0, 3, 2, 1))


def _shard(x, weight):
    in_maps = []
    for c in range(8):
        mb, ob = c // 2, c % 2
        in_maps.append(
            {
                "xP": _pack_x(x[mb * MSH : (mb + 1) * MSH, :]),
                "wT": np.ascontiguousarray(
                    weight[ob * OSH : (ob + 1) * OSH, :].T
                ),
            }
        )
    return in_maps


def _gather(results):
    out = np.empty((N, O), dtype=np.float32)
    for c in range(8):
        mb, ob = c // 2, c % 2
        out[mb * MSH : (mb + 1) * MSH, ob * OSH : (ob + 1) * OSH] = results[c][
            "out"
        ]
    return out


def kernel(x: np.ndarray, weight: np.ndarray) -> np.ndarray:
    x = np.asarray(x, dtype=np.float32)
    weight = np.asarray(weight, dtype=np.float32)
    res = _run(_shard(x, weight))
    return _gather(res.results)
